# revision 4
# baseline (speedup 1.0000x reference)
"""GroupRouter (MoE routing) kernel for 8 Trainium2 NeuronCores.

Reference computation (per token):
    xn = LayerNorm(x) * gamma + beta
    logits = xn @ W.T + b            (N=64 experts)
    w = softmax(logits)
    top-k (k=2) of w -> sparse_w (renormalized), indices
    lb_loss = sum(mean_w * log(mean_w + eps)) + 0.1 * relu(max(ema_load) - thr)

Device strategy (data parallel over tokens, 8 cores, fp32 end to end):
    logits = invsig_t * ((x @ Wg^T) - mu_t * s) + c
  with Wg = W * gamma, s = sum_d Wg, c = W @ beta + b folded on host (tiny).
  Per 128-token tile: DMA x tile -> ACT Square+accum (sum x^2) ->
  PE transpose of x chunks (fp32, bit-exact) -> PSUM->SBUF copy ->
  PE fp32 matmul against [Wg^T | 1/D] -> DVE/ACT softmax + top-k via
  max8/max_index. mean_w partials and the scalar lb_loss finish on host
  (the "all-reduce only mean_w / lb_loss" part of the sharding hint).
"""

import os
import sys

import numpy as np

for _p in ("/opt/trn_rl_repo", "/root/.axon_site/_ro/trn_rl_repo"):
    if os.path.isdir(_p) and _p not in sys.path:
        sys.path.insert(0, _p)

import concourse.bacc as bacc
import concourse.tile as tile
import concourse.mybir as mybir
from concourse.bass_utils import run_bass_kernel_spmd
from concourse.masks import make_identity

LN_EPS = 1e-5
RENORM_EPS = 1e-8
ANTICIRCLE_WEIGHT = 0.1

N_CORES = 8
P = 128          # partitions = tokens per tile
D = 2048         # model dim
N = 64           # experts
NCH = D // P     # 16 contraction chunks
G = 4            # tiles per softmax group
AF = mybir.ActivationFunctionType
ALU = mybir.AluOpType

# every n-th PSUM->SBUF half-tile copy goes to ACT (0 => all on DVE)
ACT_COPY_EVERY = 3


def build_program(ntiles: int, k: int, use_c: bool, repeat: int = 1):
    """Per-core SPMD program over ntiles tiles of 128 tokens."""
    tok = ntiles * P
    HALF = D // 2
    nc = bacc.Bacc(trn_type="TRN2", target_bir_lowering=False, debug=False)

    x_d = nc.dram_tensor("x", [tok, D], mybir.dt.float32, kind="ExternalInput")
    wstk_d = nc.dram_tensor(
        "wstk", [P, NCH * (N + 1)], mybir.dt.float32, kind="ExternalInput"
    )
    srow_d = nc.dram_tensor("srow", [P, N], mybir.dt.float32, kind="ExternalInput")
    crow_d = nc.dram_tensor("crow", [P, N], mybir.dt.float32, kind="ExternalInput")

    sparse_d = nc.dram_tensor(
        "sparse", [P, ntiles, N], mybir.dt.float32, kind="ExternalOutput"
    )
    idx_d = nc.dram_tensor("idx", [P, ntiles, k], mybir.dt.int32, kind="ExternalOutput")
    wsum_d = nc.dram_tensor("wsum", [P, N], mybir.dt.float32, kind="ExternalOutput")

    with tile.TileContext(nc) as tc:
        with (
            tc.tile_pool(name="const", bufs=1) as const,
            tc.tile_pool(name="xin", bufs=3) as xin,
            tc.tile_pool(name="xtp", bufs=2) as xtp,
            tc.tile_pool(name="xsqp", bufs=2) as xsqp,
            tc.tile_pool(name="stats", bufs=1) as stats,
            tc.tile_pool(name="psT", bufs=2, space="PSUM") as psT,
            tc.tile_pool(name="psY", bufs=4, space="PSUM") as psY,
        ):
            ident = const.tile([P, P], mybir.dt.float32)
            make_identity(nc, ident)
            eps_t = const.tile([P, 1], mybir.dt.float32)
            nc.vector.memset(eps_t, LN_EPS)
            wstk = const.tile([P, NCH, N + 1], mybir.dt.float32)
            nc.sync.dma_start(
                out=wstk, in_=wstk_d.ap().rearrange("p (c n) -> p c n", n=N + 1)
            )
            s_t = const.tile([P, N], mybir.dt.float32)
            nc.sync.dma_start(out=s_t, in_=srow_d.ap())
            c_t = None
            if use_c:
                c_t = const.tile([P, N], mybir.dt.float32)
                nc.sync.dma_start(out=c_t, in_=crow_d.ap())

            # persistent per-core stats / results
            ssq = stats.tile([P, ntiles], mybir.dt.float32)
            mu = stats.tile([P, ntiles], mybir.dt.float32)
            nivs = stats.tile([P, ntiles], mybir.dt.float32)  # -1/sigma
            scr = stats.tile([P, ntiles], mybir.dt.float32)
            scr2 = stats.tile([P, ntiles], mybir.dt.float32)
            rmin = stats.tile([P, ntiles], mybir.dt.float32)
            rr = stats.tile([P, ntiles], mybir.dt.float32)    # 1/sum(exp)
            rd = stats.tile([P, ntiles], mybir.dt.float32)    # 1/(topk sum+eps)
            nym_all = stats.tile([P, ntiles, N], mybir.dt.float32)  # mu*s - y
            w_all = stats.tile([P, ntiles, N], mybir.dt.float32)
            sp_all = stats.tile([P, ntiles, N], mybir.dt.float32)
            m8_all = stats.tile([P, ntiles, 8], mybir.dt.float32)
            i8_all = stats.tile([P, ntiles, 8], mybir.dt.uint32)
            idx_out = stats.tile([P, ntiles, k], mybir.dt.int32)

            for _rep in range(repeat):
                copy_i = 0
                for i in range(ntiles):
                    x_t = xin.tile([P, D], mybir.dt.float32)
                    nc.sync.dma_start(out=x_t, in_=x_d.ap()[i * P : (i + 1) * P, :])

                    # ssq[:, i] = sum_d x^2 (squared tile itself is scratch)
                    xsq = xsqp.tile([P, D], mybir.dt.float32)
                    nc.scalar.activation(
                        out=xsq, in_=x_t, func=AF.Square,
                        accum_out=ssq[:, i : i + 1],
                    )

                    # transpose the tile in 2 halves of 8 chunks each
                    xt_sb = xtp.tile([P, D], mybir.dt.float32)
                    for h in range(2):
                        xt_ps = psT.tile([P, HALF], mybir.dt.float32)
                        for cc in range(NCH // 2):
                            c0 = h * (NCH // 2) + cc
                            nc.tensor.transpose(
                                xt_ps[:, cc * P : (cc + 1) * P],
                                x_t[:, c0 * P : (c0 + 1) * P],
                                ident,
                            )
                        dst = xt_sb[:, h * HALF : (h + 1) * HALF]
                        copy_i += 1
                        if ACT_COPY_EVERY and copy_i % ACT_COPY_EVERY == 0:
                            nc.scalar.copy(out=dst, in_=xt_ps)
                        else:
                            nc.vector.tensor_copy(out=dst, in_=xt_ps)

                    # y[:, 0:64] = x @ Wg^T ; y[:, 64] = mean(x)
                    y_ps = psY.tile([P, N + 1], mybir.dt.float32)
                    for c in range(NCH):
                        nc.tensor.matmul(
                            y_ps,
                            xt_sb[:, c * P : (c + 1) * P],
                            wstk[:, c, :],
                            start=(c == 0),
                            stop=(c == NCH - 1),
                        )

                    nc.vector.tensor_copy(out=mu[:, i : i + 1], in_=y_ps[:, N : N + 1])
                    # nym = mu*s - y   (negated logit core)
                    nc.vector.scalar_tensor_tensor(
                        out=nym_all[:, i, :],
                        in0=s_t,
                        scalar=y_ps[:, N : N + 1],
                        in1=y_ps[:, 0:N],
                        op0=ALU.mult,
                        op1=ALU.subtract,
                    )

                    if (i + 1) % G == 0:
                        _group_math(nc, i + 1 - G, i + 1, k, use_c, locals())

                # mean_w partial: wsum[p, n] = sum_t w_all[p, t, n]
                wsum_t = stats.tile([P, N], mybir.dt.float32)
                nc.vector.tensor_reduce(
                    out=wsum_t,
                    in_=w_all.rearrange("p t n -> p n t"),
                    axis=mybir.AxisListType.X,
                    op=ALU.add,
                )

                nc.sync.dma_start(out=sparse_d.ap(), in_=sp_all)
                nc.sync.dma_start(out=idx_d.ap(), in_=idx_out)
                nc.sync.dma_start(out=wsum_d.ap(), in_=wsum_t)

    nc.compile()
    return nc


def _group_math(nc, g0, g1, k, use_c, env):
    """Softmax + top-k for tiles [g0, g1). Uses tiles from build_program."""
    ssq, mu, nivs = env["ssq"], env["mu"], env["nivs"]
    scr, scr2, rmin, rr, rd = env["scr"], env["scr2"], env["rmin"], env["rr"], env["rd"]
    nym_all, w_all, sp_all = env["nym_all"], env["w_all"], env["sp_all"]
    m8_all, i8_all, idx_out = env["m8_all"], env["i8_all"], env["idx_out"]
    c_t = env["c_t"]
    gs = slice(g0, g1)

    # var = ssq/D - mu^2 ; nivs = -exp(-0.5 * ln(var + eps))
    nc.vector.tensor_tensor(out=scr[:, gs], in0=mu[:, gs], in1=mu[:, gs], op=ALU.mult)
    nc.vector.scalar_tensor_tensor(
        out=scr2[:, gs], in0=ssq[:, gs], scalar=1.0 / D, in1=scr[:, gs],
        op0=ALU.mult, op1=ALU.subtract,
    )
    nc.scalar.activation(out=scr[:, gs], in_=scr2[:, gs], func=AF.Ln, bias=env["eps_t"])
    nc.scalar.activation(out=scr2[:, gs], in_=scr[:, gs], func=AF.Exp, scale=-0.5)
    nc.vector.tensor_scalar(
        out=nivs[:, gs], in0=scr2[:, gs], scalar1=-1.0, scalar2=None, op0=ALU.mult
    )

    if not use_c:
        # w = exp((ymu - max(ymu)) * invsig), via negated nym
        nc.vector.tensor_reduce(
            out=rmin[:, gs], in_=nym_all[:, gs, :], axis=mybir.AxisListType.X,
            op=ALU.min,
        )
        for t in range(g0, g1):
            nc.vector.tensor_scalar(
                out=w_all[:, t, :], in0=nym_all[:, t, :],
                scalar1=rmin[:, t : t + 1], scalar2=nivs[:, t : t + 1],
                op0=ALU.subtract, op1=ALU.mult,
            )
    else:
        # logits = nym * nivs + c ; subtract rowmax
        for t in range(g0, g1):
            nc.vector.tensor_scalar(
                out=w_all[:, t, :], in0=nym_all[:, t, :],
                scalar1=nivs[:, t : t + 1], scalar2=None, op0=ALU.mult,
            )
        nc.vector.tensor_tensor(
            out=w_all[:, gs, :], in0=w_all[:, gs, :],
            in1=c_t.rearrange("p n -> p 1 n").to_broadcast([P, g1 - g0, N]),
            op=ALU.add,
        )
        nc.vector.tensor_reduce(
            out=rmin[:, gs], in_=w_all[:, gs, :], axis=mybir.AxisListType.X,
            op=ALU.max, negate=True,
        )
        for t in range(g0, g1):
            nc.vector.tensor_scalar(
                out=w_all[:, t, :], in0=w_all[:, t, :],
                scalar1=rmin[:, t : t + 1], scalar2=None, op0=ALU.add,
            )

    for t in range(g0, g1):
        nc.scalar.activation(
            out=w_all[:, t, :], in_=w_all[:, t, :], func=AF.Exp,
            accum_out=scr[:, t : t + 1],
        )
    nc.vector.reciprocal(out=rr[:, gs], in_=scr[:, gs])
    for t in range(g0, g1):
        nc.vector.tensor_scalar(
            out=w_all[:, t, :], in0=w_all[:, t, :],
            scalar1=rr[:, t : t + 1], scalar2=None, op0=ALU.mult,
        )

    for t in range(g0, g1):
        nc.vector.max(out=m8_all[:, t, :], in_=w_all[:, t, :])
        nc.vector.max_index(
            out=i8_all[:, t, :], in_max=m8_all[:, t, :], in_values=w_all[:, t, :]
        )

    # rd = 1 / (sum of top-k + eps)
    if k == 1:
        nc.vector.tensor_scalar(
            out=scr[:, gs], in0=m8_all[:, gs, 0], scalar1=RENORM_EPS, scalar2=None,
            op0=ALU.add,
        )
    else:
        nc.vector.tensor_reduce(
            out=scr[:, gs], in_=m8_all[:, gs, 0:k], axis=mybir.AxisListType.X,
            op=ALU.add,
        )
        nc.vector.tensor_scalar(
            out=scr[:, gs], in0=scr[:, gs], scalar1=RENORM_EPS, scalar2=None,
            op0=ALU.add,
        )
    nc.vector.reciprocal(out=rd[:, gs], in_=scr[:, gs])

    for t in range(g0, g1):
        # mask = (w >= kth max) ; sparse = (w * rd) * mask
        nc.vector.tensor_scalar(
            out=sp_all[:, t, :], in0=w_all[:, t, :],
            scalar1=m8_all[:, t, k - 1 : k], scalar2=None, op0=ALU.is_ge,
        )
        nc.vector.scalar_tensor_tensor(
            out=sp_all[:, t, :], in0=w_all[:, t, :], scalar=rd[:, t : t + 1],
            in1=sp_all[:, t, :], op0=ALU.mult, op1=ALU.mult,
        )

    nc.vector.tensor_copy(out=idx_out[:, gs, :], in_=i8_all[:, gs, 0:k])


_PROGRAMS: dict = {}


def _get_program(ntiles: int, k: int, use_c: bool):
    key = (ntiles, k, use_c)
    if key not in _PROGRAMS:
        _PROGRAMS[key] = build_program(ntiles, k, use_c)
    return _PROGRAMS[key]


def kernel(**inputs):
    x = np.asarray(inputs["x"], dtype=np.float32)
    W = np.asarray(inputs["W"], dtype=np.float32)
    b = np.asarray(inputs["b"], dtype=np.float32)
    gamma = np.asarray(inputs["gamma"], dtype=np.float32)
    beta = np.asarray(inputs["beta"], dtype=np.float32)
    ema_load = np.asarray(inputs["ema_load"], dtype=np.float32)
    top_k = int(np.asarray(inputs["top_k"]))

    B, T, Dd = x.shape
    n_exp = W.shape[0]
    assert Dd == D and n_exp == N, (x.shape, W.shape)
    BT = B * T
    k_ref = min(top_k, n_exp)          # what the reference uses
    k_dev = min(max(k_ref, 1), 8)      # device computes 1..8 slots

    # ---- host param folding (O(N*D), tiny) ----
    Wg = W * gamma[None, :]
    s = Wg.sum(axis=1).astype(np.float32)
    c = (W @ beta + b).astype(np.float32)
    use_c = bool(np.any(c != 0.0))

    wstk = np.empty((P, NCH, N + 1), dtype=np.float32)
    wstk[:, :, :N] = Wg.T.reshape(NCH, P, N).transpose(1, 0, 2)
    wstk[:, :, N] = np.float32(1.0 / Dd)
    wstk = np.ascontiguousarray(wstk.reshape(P, NCH * (N + 1)))
    srow = np.ascontiguousarray(np.broadcast_to(s, (P, N)))
    crow = np.ascontiguousarray(np.broadcast_to(c, (P, N)))

    tok_per_core = BT // N_CORES
    ntiles = tok_per_core // P
    assert tok_per_core % P == 0

    nc = _get_program(ntiles, k_dev, use_c)

    xs = x.reshape(BT, Dd)
    in_maps = [
        {
            "x": xs[i * tok_per_core : (i + 1) * tok_per_core],
            "wstk": wstk,
            "srow": srow,
            "crow": crow,
        }
        for i in range(N_CORES)
    ]
    res = run_bass_kernel_spmd(nc, in_maps, list(range(N_CORES)))

    sparse = np.empty((BT, N), dtype=np.float32)
    idx = np.empty((BT, k_dev), dtype=np.int32)
    wsum = np.zeros(N, dtype=np.float64)
    for i, r in enumerate(res.results):
        sl = slice(i * tok_per_core, (i + 1) * tok_per_core)
        # device layout (P, ntiles, ...) -> token-major
        sparse[sl] = r["sparse"].transpose(1, 0, 2).reshape(tok_per_core, N)
        idx[sl] = r["idx"].transpose(1, 0, 2).reshape(tok_per_core, k_dev)
        wsum += r["wsum"].astype(np.float64).sum(axis=0)

    mean_w = (wsum / BT).astype(np.float32)
    lb = np.float32(np.sum(mean_w * np.log(mean_w + np.float32(RENORM_EPS))))
    uniform = 1.0 / n_exp
    margin = min(0.15, (1.0 - uniform) * 0.3)
    threshold = uniform + margin
    penalty = max(float(ema_load.max()) - threshold, 0.0)
    lb_loss = np.float32(lb + ANTICIRCLE_WEIGHT * penalty)

    if k_ref == 0:
        return (
            np.zeros((B, T, N), dtype=np.float32),
            np.zeros((B, T, 0), dtype=np.int32),
            lb_loss,
        )
    return (
        sparse.reshape(B, T, N),
        idx[:, :k_ref].reshape(B, T, k_ref),
        lb_loss,
    )


# revision 8
# speedup vs baseline: 1.2649x; 1.2649x over previous
"""GroupRouter (MoE routing) kernel for 8 Trainium2 NeuronCores.

Reference computation (per token):
    xn = LayerNorm(x) * gamma + beta
    logits = xn @ W.T + b            (N=64 experts)
    w = softmax(logits)
    top-k (k=2) of w -> sparse_w (renormalized), indices
    lb_loss = sum(mean_w * log(mean_w + eps)) + 0.1 * relu(max(ema_load) - thr)

Device strategy (data parallel over tokens, 8 cores, fp32 end to end):
    logits = invsig_t * ((x @ Wg^T) - mu_t * s) + c
  with Wg = W * gamma, s = sum_d Wg, c = W @ beta + b folded on host (tiny).
  Per 128-token tile: DMA x tile -> ACT Square+accum (sum x^2) ->
  PE transpose of x chunks (fp32, bit-exact) -> PSUM->SBUF copy ->
  PE fp32 matmul against [Wg^T | 1/D] -> DVE/ACT softmax + top-k via
  max8/max_index. mean_w partials and the scalar lb_loss finish on host
  (the "all-reduce only mean_w / lb_loss" part of the sharding hint).
"""

import os
import sys

import numpy as np

for _p in ("/opt/trn_rl_repo", "/root/.axon_site/_ro/trn_rl_repo"):
    if os.path.isdir(_p) and _p not in sys.path:
        sys.path.insert(0, _p)

import concourse.bacc as bacc
import concourse.tile as tile
import concourse.mybir as mybir
from concourse.bass_utils import run_bass_kernel_spmd
from concourse.masks import make_identity

# Route every activation we use (Square/Ln/Exp/Copy) to the one table set
# that holds them all, so ACT does a single table load instead of thrashing
# between exp_and_others and natural_log_exp_and_others every group.
_orig_get_tables = bacc.get_activation_tables


def _pinned_tables(arch):
    tables = _orig_get_tables(arch)
    ours = {
        mybir.ActivationFunctionType.Square,
        mybir.ActivationFunctionType.Ln,
        mybir.ActivationFunctionType.Exp,
        mybir.ActivationFunctionType.Copy,
        mybir.ActivationFunctionType.Identity,
    }
    home = None
    for name, fns in tables.items():
        if ours <= fns:
            home = name
            break
    if home is not None:
        for name, fns in tables.items():
            if name != home:
                tables[name] = fns - ours
    return tables


bacc.get_activation_tables = _pinned_tables

LN_EPS = 1e-5
RENORM_EPS = 1e-8
ANTICIRCLE_WEIGHT = 0.1

N_CORES = 8
P = 128          # partitions = tokens per tile
D = 2048         # model dim
N = 64           # experts
NCH = D // P     # 16 contraction chunks
G = 2            # tiles per softmax group
AF = mybir.ActivationFunctionType
ALU = mybir.AluOpType

# every n-th PSUM->SBUF half-tile copy goes to ACT (0 => all on DVE)
ACT_COPY_EVERY = 3


def build_program(ntiles: int, k: int, use_c: bool, repeat: int = 1):
    """Per-core SPMD program over ntiles tiles of 128 tokens."""
    tok = ntiles * P
    HALF = D // 2
    nc = bacc.Bacc(trn_type="TRN2", target_bir_lowering=False, debug=False)

    x_d = nc.dram_tensor("x", [tok, D], mybir.dt.float32, kind="ExternalInput")
    wstk_d = nc.dram_tensor(
        "wstk", [P, NCH * (N + 1)], mybir.dt.float32, kind="ExternalInput"
    )
    srow_d = nc.dram_tensor("srow", [P, N], mybir.dt.float32, kind="ExternalInput")
    crow_d = nc.dram_tensor("crow", [P, N], mybir.dt.float32, kind="ExternalInput")

    sparse_d = nc.dram_tensor(
        "sparse", [P, ntiles, N], mybir.dt.float32, kind="ExternalOutput"
    )
    idx_d = nc.dram_tensor("idx", [P, ntiles, k], mybir.dt.int32, kind="ExternalOutput")
    wsum_d = nc.dram_tensor("wsum", [P, N], mybir.dt.float32, kind="ExternalOutput")

    with tile.TileContext(nc) as tc:
        with (
            tc.tile_pool(name="const", bufs=1) as const,
            tc.tile_pool(name="xin", bufs=4) as xin,
            tc.tile_pool(name="xtp", bufs=3) as xtp,
            tc.tile_pool(name="xsqp", bufs=2) as xsqp,
            tc.tile_pool(name="stats", bufs=1) as stats,
            tc.tile_pool(name="psT", bufs=3, space="PSUM") as psT,
            tc.tile_pool(name="psY", bufs=2, space="PSUM") as psY,
        ):
            ident = const.tile([P, P], mybir.dt.float32)
            make_identity(nc, ident)
            eps_t = const.tile([P, 1], mybir.dt.float32)
            nc.vector.memset(eps_t, LN_EPS)
            wstk = const.tile([P, NCH, N + 1], mybir.dt.float32)
            nc.sync.dma_start(
                out=wstk, in_=wstk_d.ap().rearrange("p (c n) -> p c n", n=N + 1)
            )
            s_t = const.tile([P, N], mybir.dt.float32)
            nc.sync.dma_start(out=s_t, in_=srow_d.ap())
            c_t = None
            if use_c:
                c_t = const.tile([P, N], mybir.dt.float32)
                nc.sync.dma_start(out=c_t, in_=crow_d.ap())

            # persistent per-core stats / results
            ssq = stats.tile([P, ntiles], mybir.dt.float32)
            mu = stats.tile([P, ntiles], mybir.dt.float32)
            nivs = stats.tile([P, ntiles], mybir.dt.float32)  # -1/sigma
            scr = stats.tile([P, ntiles], mybir.dt.float32)
            scr2 = stats.tile([P, ntiles], mybir.dt.float32)
            rmin = stats.tile([P, ntiles], mybir.dt.float32)
            rr = stats.tile([P, ntiles], mybir.dt.float32)    # 1/sum(exp)
            rd = stats.tile([P, ntiles], mybir.dt.float32)    # 1/(topk sum+eps)
            nym_all = stats.tile([P, ntiles, N], mybir.dt.float32)  # mu*s - y
            w_all = stats.tile([P, ntiles, N], mybir.dt.float32)
            sp_all = stats.tile([P, ntiles, N], mybir.dt.float32)
            m8_all = stats.tile([P, ntiles, 8], mybir.dt.float32)
            i8_all = stats.tile([P, ntiles, 8], mybir.dt.uint32)
            idx_out = stats.tile([P, ntiles, k], mybir.dt.int32)

            for _rep in range(repeat):
                copy_i = 0
                for i in range(ntiles):
                    x_t = xin.tile([P, D], mybir.dt.float32)
                    for hh in range(2):
                        nc.sync.dma_start(
                            out=x_t[:, hh * HALF : (hh + 1) * HALF],
                            in_=x_d.ap()[i * P : (i + 1) * P, hh * HALF : (hh + 1) * HALF],
                        )

                    # ssq[:, i] = sum_d x^2 (squared tile itself is scratch)
                    xsq = xsqp.tile([P, D], mybir.dt.float32)
                    nc.scalar.activation(
                        out=xsq, in_=x_t, func=AF.Square,
                        accum_out=ssq[:, i : i + 1],
                    )

                    # transpose the tile in 2 halves of 8 chunks each
                    xt_sb = xtp.tile([P, D], mybir.dt.float32)
                    for h in range(2):
                        xt_ps = psT.tile([P, HALF], mybir.dt.float32)
                        for cc in range(NCH // 2):
                            c0 = h * (NCH // 2) + cc
                            nc.tensor.transpose(
                                xt_ps[:, cc * P : (cc + 1) * P],
                                x_t[:, c0 * P : (c0 + 1) * P],
                                ident,
                            )
                        dst = xt_sb[:, h * HALF : (h + 1) * HALF]
                        copy_i += 1
                        if ACT_COPY_EVERY and copy_i % ACT_COPY_EVERY == 0:
                            nc.scalar.copy(out=dst, in_=xt_ps)
                        else:
                            nc.vector.tensor_copy(out=dst, in_=xt_ps)

                    # y[:, 0:64] = x @ Wg^T ; y[:, 64] = mean(x)
                    y_ps = psY.tile([P, N + 1], mybir.dt.float32)
                    for c in range(NCH):
                        nc.tensor.matmul(
                            y_ps,
                            xt_sb[:, c * P : (c + 1) * P],
                            wstk[:, c, :],
                            start=(c == 0),
                            stop=(c == NCH - 1),
                        )

                    nc.vector.tensor_copy(out=mu[:, i : i + 1], in_=y_ps[:, N : N + 1])
                    # nym = mu*s - y   (negated logit core)
                    nc.vector.scalar_tensor_tensor(
                        out=nym_all[:, i, :],
                        in0=s_t,
                        scalar=y_ps[:, N : N + 1],
                        in1=y_ps[:, 0:N],
                        op0=ALU.mult,
                        op1=ALU.subtract,
                    )

                    if (i + 1) % G == 0:
                        env = locals()
                        env["sparse_d"] = sparse_d
                        _group_math(nc, i + 1 - G, i + 1, k, use_c, env)

                # mean_w partial: wsum[p, n] = sum_t w_all[p, t, n]
                wsum_t = stats.tile([P, N], mybir.dt.float32)
                nc.vector.tensor_reduce(
                    out=wsum_t,
                    in_=w_all.rearrange("p t n -> p n t"),
                    axis=mybir.AxisListType.X,
                    op=ALU.add,
                )

                nc.sync.dma_start(out=idx_d.ap(), in_=idx_out)
                nc.sync.dma_start(out=wsum_d.ap(), in_=wsum_t)

    nc.compile()
    return nc


def _group_math(nc, g0, g1, k, use_c, env):
    """Softmax + top-k for tiles [g0, g1). Uses tiles from build_program."""
    ssq, mu, nivs = env["ssq"], env["mu"], env["nivs"]
    scr, scr2, rmin, rr, rd = env["scr"], env["scr2"], env["rmin"], env["rr"], env["rd"]
    nym_all, w_all, sp_all = env["nym_all"], env["w_all"], env["sp_all"]
    m8_all, i8_all, idx_out = env["m8_all"], env["i8_all"], env["idx_out"]
    c_t = env["c_t"]
    gs = slice(g0, g1)

    # var = ssq/D - mu^2 ; nivs = -exp(-0.5 * ln(var + eps))
    nc.vector.tensor_tensor(out=scr[:, gs], in0=mu[:, gs], in1=mu[:, gs], op=ALU.mult)
    nc.vector.scalar_tensor_tensor(
        out=scr2[:, gs], in0=ssq[:, gs], scalar=1.0 / D, in1=scr[:, gs],
        op0=ALU.mult, op1=ALU.subtract,
    )
    nc.scalar.activation(out=scr[:, gs], in_=scr2[:, gs], func=AF.Ln, bias=env["eps_t"])
    nc.scalar.activation(out=scr2[:, gs], in_=scr[:, gs], func=AF.Exp, scale=-0.5)
    nc.vector.tensor_scalar(
        out=nivs[:, gs], in0=scr2[:, gs], scalar1=-1.0, scalar2=None, op0=ALU.mult
    )

    if not use_c:
        # w = exp((ymu - max(ymu)) * invsig), via negated nym
        nc.vector.tensor_reduce(
            out=rmin[:, gs], in_=nym_all[:, gs, :], axis=mybir.AxisListType.X,
            op=ALU.min,
        )
        for t in range(g0, g1):
            nc.vector.tensor_scalar(
                out=w_all[:, t, :], in0=nym_all[:, t, :],
                scalar1=rmin[:, t : t + 1], scalar2=nivs[:, t : t + 1],
                op0=ALU.subtract, op1=ALU.mult,
            )
    else:
        # logits = nym * nivs + c ; subtract rowmax
        for t in range(g0, g1):
            nc.vector.tensor_scalar(
                out=w_all[:, t, :], in0=nym_all[:, t, :],
                scalar1=nivs[:, t : t + 1], scalar2=None, op0=ALU.mult,
            )
        nc.vector.tensor_tensor(
            out=w_all[:, gs, :], in0=w_all[:, gs, :],
            in1=c_t.rearrange("p n -> p 1 n").to_broadcast([P, g1 - g0, N]),
            op=ALU.add,
        )
        nc.vector.tensor_reduce(
            out=rmin[:, gs], in_=w_all[:, gs, :], axis=mybir.AxisListType.X,
            op=ALU.max, negate=True,
        )
        for t in range(g0, g1):
            nc.vector.tensor_scalar(
                out=w_all[:, t, :], in0=w_all[:, t, :],
                scalar1=rmin[:, t : t + 1], scalar2=None, op0=ALU.add,
            )

    for t in range(g0, g1):
        nc.scalar.activation(
            out=w_all[:, t, :], in_=w_all[:, t, :], func=AF.Exp,
            accum_out=scr[:, t : t + 1],
        )
    nc.vector.reciprocal(out=rr[:, gs], in_=scr[:, gs])
    for t in range(g0, g1):
        nc.vector.tensor_scalar(
            out=w_all[:, t, :], in0=w_all[:, t, :],
            scalar1=rr[:, t : t + 1], scalar2=None, op0=ALU.mult,
        )

    for t in range(g0, g1):
        nc.vector.max(out=m8_all[:, t, :], in_=w_all[:, t, :])
        nc.vector.max_index(
            out=i8_all[:, t, :], in_max=m8_all[:, t, :], in_values=w_all[:, t, :]
        )

    # rd = 1 / (sum of top-k + eps)
    if k == 1:
        nc.vector.tensor_scalar(
            out=scr[:, gs], in0=m8_all[:, gs, 0], scalar1=RENORM_EPS, scalar2=None,
            op0=ALU.add,
        )
    else:
        nc.vector.tensor_reduce(
            out=scr[:, gs], in_=m8_all[:, gs, 0:k], axis=mybir.AxisListType.X,
            op=ALU.add,
        )
        nc.vector.tensor_scalar(
            out=scr[:, gs], in0=scr[:, gs], scalar1=RENORM_EPS, scalar2=None,
            op0=ALU.add,
        )
    nc.vector.reciprocal(out=rd[:, gs], in_=scr[:, gs])

    for t in range(g0, g1):
        # mask = (w >= kth max) ; sparse = (w * rd) * mask
        nc.vector.tensor_scalar(
            out=sp_all[:, t, :], in0=w_all[:, t, :],
            scalar1=m8_all[:, t, k - 1 : k], scalar2=None, op0=ALU.is_ge,
        )
        nc.vector.scalar_tensor_tensor(
            out=sp_all[:, t, :], in0=w_all[:, t, :], scalar=rd[:, t : t + 1],
            in1=sp_all[:, t, :], op0=ALU.mult, op1=ALU.mult,
        )

    nc.vector.tensor_copy(out=idx_out[:, gs, :], in_=i8_all[:, gs, 0:k])
    nc.sync.dma_start(
        out=env["sparse_d"].ap()[:, gs, :], in_=env["sp_all"][:, gs, :]
    )


_PROGRAMS: dict = {}


def _get_program(ntiles: int, k: int, use_c: bool):
    key = (ntiles, k, use_c)
    if key not in _PROGRAMS:
        _PROGRAMS[key] = build_program(ntiles, k, use_c)
    return _PROGRAMS[key]


def kernel(**inputs):
    x = np.asarray(inputs["x"], dtype=np.float32)
    W = np.asarray(inputs["W"], dtype=np.float32)
    b = np.asarray(inputs["b"], dtype=np.float32)
    gamma = np.asarray(inputs["gamma"], dtype=np.float32)
    beta = np.asarray(inputs["beta"], dtype=np.float32)
    ema_load = np.asarray(inputs["ema_load"], dtype=np.float32)
    top_k = int(np.asarray(inputs["top_k"]))

    B, T, Dd = x.shape
    n_exp = W.shape[0]
    assert Dd == D and n_exp == N, (x.shape, W.shape)
    BT = B * T
    k_ref = min(top_k, n_exp)          # what the reference uses
    k_dev = min(max(k_ref, 1), 8)      # device computes 1..8 slots

    # ---- host param folding (O(N*D), tiny) ----
    Wg = W * gamma[None, :]
    s = Wg.sum(axis=1).astype(np.float32)
    c = (W @ beta + b).astype(np.float32)
    use_c = bool(np.any(c != 0.0))

    wstk = np.empty((P, NCH, N + 1), dtype=np.float32)
    wstk[:, :, :N] = Wg.T.reshape(NCH, P, N).transpose(1, 0, 2)
    wstk[:, :, N] = np.float32(1.0 / Dd)
    wstk = np.ascontiguousarray(wstk.reshape(P, NCH * (N + 1)))
    srow = np.ascontiguousarray(np.broadcast_to(s, (P, N)))
    crow = np.ascontiguousarray(np.broadcast_to(c, (P, N)))

    tok_per_core = BT // N_CORES
    ntiles = tok_per_core // P
    assert tok_per_core % P == 0

    nc = _get_program(ntiles, k_dev, use_c)

    xs = x.reshape(BT, Dd)
    in_maps = [
        {
            "x": xs[i * tok_per_core : (i + 1) * tok_per_core],
            "wstk": wstk,
            "srow": srow,
            "crow": crow,
        }
        for i in range(N_CORES)
    ]
    res = run_bass_kernel_spmd(nc, in_maps, list(range(N_CORES)))

    sparse = np.empty((BT, N), dtype=np.float32)
    idx = np.empty((BT, k_dev), dtype=np.int32)
    wsum = np.zeros(N, dtype=np.float64)
    for i, r in enumerate(res.results):
        sl = slice(i * tok_per_core, (i + 1) * tok_per_core)
        # device layout (P, ntiles, ...) -> token-major
        sparse[sl] = r["sparse"].transpose(1, 0, 2).reshape(tok_per_core, N)
        idx[sl] = r["idx"].transpose(1, 0, 2).reshape(tok_per_core, k_dev)
        wsum += r["wsum"].astype(np.float64).sum(axis=0)

    mean_w = (wsum / BT).astype(np.float32)
    lb = np.float32(np.sum(mean_w * np.log(mean_w + np.float32(RENORM_EPS))))
    uniform = 1.0 / n_exp
    margin = min(0.15, (1.0 - uniform) * 0.3)
    threshold = uniform + margin
    penalty = max(float(ema_load.max()) - threshold, 0.0)
    lb_loss = np.float32(lb + ANTICIRCLE_WEIGHT * penalty)

    if k_ref == 0:
        return (
            np.zeros((B, T, N), dtype=np.float32),
            np.zeros((B, T, 0), dtype=np.int32),
            lb_loss,
        )
    return (
        sparse.reshape(B, T, N),
        idx[:, :k_ref].reshape(B, T, k_ref),
        lb_loss,
    )
